# revision 22
# baseline (speedup 1.0000x reference)
"""BlurPool3D Trainium2 kernel (8 cores, depthwise 3x3x3 blur, stride 2).

x[2, 64, 64, 96, 96] f32 -> y[2, 64, 32, 48, 48] f32. Rank-1 separable
filter (binomial [1,2,1]^3 / 64).

Strategy (v2 — subsample-first pass order):
  - Shard the 128 (n, c) pairs across 8 cores: 16 per core, 8 blocks of
    2 channels. Channels are independent (depthwise) -> no collectives.
  - Per block, SBUF partitions = (2 nc x 64 d) = 128; free = (h, w).
    Two input tiles per block: rows 0-47 and rows 47-95 (the two output
    h-halves need x rows [-1..47] / [47..95]).
  - D and H blur+subsample run FIRST, fused in one matmul family:
    lhsT = block-diagonal D-band matrix [128, 64] scaled by hvec[k]*w0,
    rhs = x rows (2h'-1+k) strided; 3 taps (k) accumulate in PSUM.
    float32r matmuls run at ~1 cycle/row (vs 4 for plain fp32) for
    N >= 256. Output: [64 (ncl,d'), 5h' x 96w] per PSUM bank; the two
    h-halves go to PE column groups 0/1 -> PSUM partitions 0-63/64-127.
    The h'=0 top edge (zero pad) is a shortened k=0 matmul; d-edges live
    in the band matrix. No memsets, no halo exchange.
  - W blur+subsample runs LAST on VectorE, on data already 4x smaller
    than x: per chunk two strided STTs (out = left + r1*mid + r2*right)
    reading PSUM directly, writing the SBUF out tile, plus a tiny
    2-op w'=0 edge column fix.
  - DMA: per block two ~2.4 MB input DMAs (sync=SP ring / scalar=ACT
    ring) and one 590 KB 128-partition output DMA on gpsimd (SWDGE) so
    output issue never head-blocks the HWDGE input streams.
"""

import os
import sys

for _p in ("/opt/trn_rl_repo",):
    if _p not in sys.path and os.path.isdir(_p):
        sys.path.insert(0, _p)

import numpy as np

N, C, D, H, W = 2, 64, 64, 96, 96
DO, HO, WO = 32, 48, 48
NCORES = 8
NC_PER_CORE = (N * C) // NCORES  # 16
BLOCKS = NC_PER_CORE // 2  # 8 blocks of 2 channels each

# h' chunks per h-half: PSUM bank = 512 f32 -> at most 5 rows of 96
CHUNKS = [(0, 5), (5, 5), (10, 5), (15, 5), (20, 4)]

_PROGRAM_CACHE = {}


def _rank1_factors(filt):
    """Per-channel rank-1 factorization filt[c,0] = outer(d, h, w)."""
    dvec = np.empty((C, 3), np.float64)
    hvec = np.empty((C, 3), np.float64)
    wvec = np.empty((C, 3), np.float64)
    for c in range(C):
        T = filt[c, 0].astype(np.float64)
        idx = np.unravel_index(np.argmax(np.abs(T)), T.shape)
        i0, j0, k0 = idx
        piv = T[i0, j0, k0]
        if piv == 0.0:
            dvec[c] = hvec[c] = wvec[c] = 0.0
            continue
        dvec[c] = T[:, j0, k0]
        hvec[c] = T[i0, :, k0] / piv
        wvec[c] = T[i0, j0, :] / piv
        recon = np.einsum("i,j,k->ijk", dvec[c], hvec[c], wvec[c])
        resid = np.abs(recon - T).max()
        if resid > 1e-6 * max(np.abs(T).max(), 1e-30):
            raise ValueError(f"filter channel {c} is not rank-1 (resid {resid})")
    return dvec, hvec, wvec


def _build_program(uniform):
    import concourse.bacc as bacc
    import concourse.mybir as mybir
    from concourse import tile

    dt = mybir.dt
    nc = bacc.Bacc("TRN2", target_bir_lowering=False, debug=False,
                   num_devices=NCORES)

    nbm = 1 if uniform else BLOCKS
    x = nc.dram_tensor("x", [NC_PER_CORE, D, H * W], dt.float32,
                       kind="ExternalInput")
    bmat = nc.dram_tensor("bmat", [128, nbm * 3 * 64], dt.bfloat16,
                          kind="ExternalInput")
    wtaps = nc.dram_tensor("wtaps", [128, 2 * nbm], dt.float32,
                           kind="ExternalInput")
    # block-major output layout: [block, h-half, ncl, d', 24*48] so each
    # block's output is one contiguous [128, 1152] DMA; host reassembles
    y = nc.dram_tensor("y", [BLOCKS, 2, 2, DO, (HO // 2) * WO], dt.float32,
                       kind="ExternalOutput")

    mult = mybir.AluOpType.mult
    add = mybir.AluOpType.add

    with tile.TileContext(nc) as tc:
        with tc.tile_pool(name="const", bufs=1) as cpool, \
             tc.tile_pool(name="xa", bufs=5) as xapool, \
             tc.tile_pool(name="xb", bufs=5) as xbpool, \
             tc.tile_pool(name="op", bufs=4) as opool, \
             tc.tile_pool(name="ps", bufs=8, space="PSUM") as pspool:
            bt = cpool.tile([128, nbm * 3 * 64], dt.bfloat16)
            wt = cpool.tile([128, 2 * nbm], dt.float32)
            nc.sync.dma_start(bt[:], bmat[:])
            nc.sync.dma_start(wt[:], wtaps[:])

            for b in range(BLOCKS):
                bi = 0 if uniform else b
                r1 = wt[:, 2 * bi:2 * bi + 1]
                r2 = wt[:, 2 * bi + 1:2 * bi + 2]
                src = x[2 * b:2 * b + 2].rearrange("a d f -> (a d) f")
                src = src.rearrange("p (h w) -> p h w", h=H)

                # xa rows r = x rows r (0..47); xb rows r = x rows 47+r.
                # f32 -> bf16 cast happens inline in the (SWDGE) DMA, so
                # HBM read traffic is unchanged but matmuls run at bf16
                # rate and SBUF input tiles halve. Big transfers keep
                # SWDGE at line rate; only the LAST block is split finer
                # so its compute can trail the end of the stream closely.
                xa = xapool.tile([128, 48, W], dt.bfloat16, tag="xa")
                xb = xbpool.tile([128, 49, W], dt.bfloat16, tag="xb")
                nc.gpsimd.dma_start(xa[:], src[:, 0:48, :])
                nc.gpsimd.dma_start(xb[:], src[:, 47:96, :])

                ot = opool.tile([128, 2 * HO // 4, WO], dt.float32)

                for h0, cnt in CHUNKS:
                    ps = pspool.tile([128, 5, W], dt.float32, tag="ps",
                                     name="ps")
                    for g in range(2):
                        xt = xa if g == 0 else xb
                        # tap order 1,2,0 so the shortened k=0 tap of
                        # (chunk 0, half 0) accumulates into rows the
                        # k=1 tap already initialized
                        for k in (1, 2, 0):
                            base = 2 * h0 - 1 + k if g == 0 else 2 * h0 + k
                            lo = 0
                            if base < 0:
                                lo = 1  # h'=0 k=0 tap is the zero pad
                                base += 2
                            rows = cnt - lo
                            rhs = xt[:, base:base + 2 * rows - 1:2, :]
                            out = ps[g * 64:(g + 1) * 64, lo:cnt, :]
                            lhsT = bt[:, (bi * 3 + k) * 64:
                                      (bi * 3 + k + 1) * 64]
                            nc.tensor.matmul(
                                out, lhsT, rhs,
                                start=(k == 1), stop=(k == 0),
                                tile_position=(0, 64 * g) if g else None)

                    # W pass: out = left + r1*mid + r2*right, stride 2.
                    # Each op reads exactly one PSUM operand (HW limit);
                    # the w'=0 edge (zero left pad) falls out naturally.
                    orows = ot[:, h0:h0 + cnt, :]
                    pv = ps[:, 0:cnt, :]
                    # mid*r1 on ScalarE (own SBUF port; a DVE tensor_scalar
                    # could enter 2-port perf mode and starve SWDGE
                    # descriptor generation for the input cast-DMAs)
                    nc.scalar.activation(
                        orows[:, :, 0:WO], pv[:, :, 0:2 * WO:2],
                        mybir.ActivationFunctionType.Identity, scale=r1)
                    nc.vector.tensor_tensor(
                        orows[:, :, 1:WO], pv[:, :, 1:2 * WO - 2:2],
                        orows[:, :, 1:WO], add)
                    nc.vector.scalar_tensor_tensor(
                        orows[:, :, 0:WO], pv[:, :, 1:2 * WO:2], r2,
                        orows[:, :, 0:WO], mult, add)

                # one 128-partition output DMA per block on the SWDGE
                # (gpsimd) path: partitions (g, ncl, d'), 4608 B each
                dst = y[b].rearrange("g a d (h w) -> (g a d) h w",
                                     h=HO // 2)
                nc.sync.dma_start(dst, ot[:])
    nc.compile()
    return nc


def kernel(x, filt):
    x = np.ascontiguousarray(np.asarray(x, dtype=np.float32))
    filt = np.asarray(filt, dtype=np.float32)
    assert x.shape == (N, C, D, H, W), x.shape

    from concourse.bass_utils import run_bass_kernel_spmd

    dvec, hvec, wvec = _rank1_factors(filt)
    w0 = wvec[:, 0].copy()
    if not (np.abs(w0) > 1e-30).all():
        raise ValueError("W-tap pivot is zero; unsupported filter")
    r1 = wvec[:, 1] / w0
    r2 = wvec[:, 2] / w0

    uniform = bool(np.all(filt == filt[:1]))
    nbm = 1 if uniform else BLOCKS
    xr = x.reshape(N * C, D, H * W)

    in_maps = []
    for core in range(NCORES):
        chans = (np.arange(NC_PER_CORE) + core * NC_PER_CORE) % C
        # band matrices: rows (ncl*64 + d), col block (bi*3 + k),
        # cols (ncl*32 + d'); value dvec[delta]*hvec[k]*w0
        bm = np.zeros((128, nbm * 3 * 64), np.float32)
        # W-pass scalars per partition (g, ncl, d')
        wtp = np.empty((128, 2 * nbm), np.float32)
        for bi in range(nbm):
            for ncl in range(2):
                c = chans[2 * bi + ncl]
                for g in range(2):
                    rows = slice(g * 64 + ncl * 32, g * 64 + ncl * 32 + 32)
                    wtp[rows, 2 * bi] = r1[c]
                    wtp[rows, 2 * bi + 1] = r2[c]
                for k in range(3):
                    col0 = (bi * 3 + k) * 64 + ncl * 32
                    for dp in range(DO):
                        for delta in range(3):
                            d = 2 * dp - 1 + delta
                            if 0 <= d < D:
                                bm[ncl * 64 + d, col0 + dp] = (
                                    dvec[c, delta] * hvec[c, k] * w0[c])
        import ml_dtypes
        in_maps.append({
            "x": np.ascontiguousarray(
                xr[core * NC_PER_CORE:(core + 1) * NC_PER_CORE]),
            "bmat": bm.astype(ml_dtypes.bfloat16),
            "wtaps": wtp,
        })

    key = ("prog", uniform)
    if key not in _PROGRAM_CACHE:
        _PROGRAM_CACHE[key] = _build_program(uniform)
    nc = _PROGRAM_CACHE[key]

    trace = bool(int(os.environ.get("BLURPOOL_TRACE", "0")))
    kwargs = {}
    if trace and os.environ.get("BLURPOOL_TRACE_DIR"):
        kwargs["tmpdir"] = os.environ["BLURPOOL_TRACE_DIR"]
    res = run_bass_kernel_spmd(nc, in_maps, core_ids=list(range(NCORES)),
                               trace=trace, **kwargs)
    if trace:
        kernel.last_result = res

    parts = []
    for r in res.results:
        yg = r["y"].reshape(BLOCKS, 2, 2, DO, HO // 2, WO)
        # (b, g, ncl, d', h', w') -> (b, ncl, d', g, h', w')
        parts.append(yg.transpose(0, 2, 3, 1, 4, 5).reshape(
            NC_PER_CORE, DO, HO, WO))
    out = np.concatenate(parts, axis=0)
    return np.ascontiguousarray(out.reshape(N, C, DO, HO, WO))


# revision 24
# speedup vs baseline: 1.0084x; 1.0084x over previous
"""BlurPool3D Trainium2 kernel (8 cores, depthwise 3x3x3 blur, stride 2).

x[2, 64, 64, 96, 96] f32 -> y[2, 64, 32, 48, 48] f32. Rank-1 separable
filter (binomial [1,2,1]^3 / 64).

Strategy (v2 — subsample-first pass order):
  - Shard the 128 (n, c) pairs across 8 cores: 16 per core, 8 blocks of
    2 channels. Channels are independent (depthwise) -> no collectives.
  - Per block, SBUF partitions = (2 nc x 64 d) = 128; free = (h, w).
    Two input tiles per block: rows 0-47 and rows 47-95 (the two output
    h-halves need x rows [-1..47] / [47..95]).
  - D and H blur+subsample run FIRST, fused in one matmul family:
    lhsT = block-diagonal D-band matrix [128, 64] scaled by hvec[k]*w0,
    rhs = x rows (2h'-1+k) strided; 3 taps (k) accumulate in PSUM.
    float32r matmuls run at ~1 cycle/row (vs 4 for plain fp32) for
    N >= 256. Output: [64 (ncl,d'), 5h' x 96w] per PSUM bank; the two
    h-halves go to PE column groups 0/1 -> PSUM partitions 0-63/64-127.
    The h'=0 top edge (zero pad) is a shortened k=0 matmul; d-edges live
    in the band matrix. No memsets, no halo exchange.
  - W blur+subsample runs LAST on VectorE, on data already 4x smaller
    than x: per chunk two strided STTs (out = left + r1*mid + r2*right)
    reading PSUM directly, writing the SBUF out tile, plus a tiny
    2-op w'=0 edge column fix.
  - DMA: per block two ~2.4 MB input DMAs (sync=SP ring / scalar=ACT
    ring) and one 590 KB 128-partition output DMA on gpsimd (SWDGE) so
    output issue never head-blocks the HWDGE input streams.
"""

import os
import sys

for _p in ("/opt/trn_rl_repo",):
    if _p not in sys.path and os.path.isdir(_p):
        sys.path.insert(0, _p)

import numpy as np

N, C, D, H, W = 2, 64, 64, 96, 96
DO, HO, WO = 32, 48, 48
NCORES = 8
NC_PER_CORE = (N * C) // NCORES  # 16
BLOCKS = NC_PER_CORE // 2  # 8 blocks of 2 channels each

# h' chunks per h-half: PSUM bank = 512 f32 -> at most 5 rows of 96
CHUNKS = [(0, 5), (5, 5), (10, 5), (15, 5), (20, 4)]

_PROGRAM_CACHE = {}


def _rank1_factors(filt):
    """Per-channel rank-1 factorization filt[c,0] = outer(d, h, w)."""
    dvec = np.empty((C, 3), np.float64)
    hvec = np.empty((C, 3), np.float64)
    wvec = np.empty((C, 3), np.float64)
    for c in range(C):
        T = filt[c, 0].astype(np.float64)
        idx = np.unravel_index(np.argmax(np.abs(T)), T.shape)
        i0, j0, k0 = idx
        piv = T[i0, j0, k0]
        if piv == 0.0:
            dvec[c] = hvec[c] = wvec[c] = 0.0
            continue
        dvec[c] = T[:, j0, k0]
        hvec[c] = T[i0, :, k0] / piv
        wvec[c] = T[i0, j0, :] / piv
        recon = np.einsum("i,j,k->ijk", dvec[c], hvec[c], wvec[c])
        resid = np.abs(recon - T).max()
        if resid > 1e-6 * max(np.abs(T).max(), 1e-30):
            raise ValueError(f"filter channel {c} is not rank-1 (resid {resid})")
    return dvec, hvec, wvec


def _build_program(uniform):
    import concourse.bacc as bacc
    import concourse.mybir as mybir
    from concourse import tile

    dt = mybir.dt
    nc = bacc.Bacc("TRN2", target_bir_lowering=False, debug=False,
                   num_devices=NCORES)

    nbm = 1 if uniform else BLOCKS
    x = nc.dram_tensor("x", [NC_PER_CORE, D, H * W], dt.float32,
                       kind="ExternalInput")
    bmat = nc.dram_tensor("bmat", [128, nbm * 3 * 64], dt.bfloat16,
                          kind="ExternalInput")
    wtaps = nc.dram_tensor("wtaps", [128, 2 * nbm], dt.float32,
                           kind="ExternalInput")
    # block-major output layout: [block, h-half, ncl, d', 24*48] so each
    # block's output is one contiguous [128, 1152] DMA; host reassembles
    y = nc.dram_tensor("y", [BLOCKS, 2, 2, DO, (HO // 2) * WO], dt.float32,
                       kind="ExternalOutput")

    mult = mybir.AluOpType.mult
    add = mybir.AluOpType.add

    with tile.TileContext(nc) as tc:
        with tc.tile_pool(name="const", bufs=1) as cpool, \
             tc.tile_pool(name="xa", bufs=5) as xapool, \
             tc.tile_pool(name="xb", bufs=5) as xbpool, \
             tc.tile_pool(name="op", bufs=4) as opool, \
             tc.tile_pool(name="ps", bufs=8, space="PSUM") as pspool:
            bt = cpool.tile([128, nbm * 3 * 64], dt.bfloat16)
            wt = cpool.tile([128, 2 * nbm], dt.float32)
            nc.sync.dma_start(bt[:], bmat[:])
            nc.sync.dma_start(wt[:], wtaps[:])

            for b in range(BLOCKS):
                bi = 0 if uniform else b
                r1 = wt[:, 2 * bi:2 * bi + 1]
                r2 = wt[:, 2 * bi + 1:2 * bi + 2]
                src = x[2 * b:2 * b + 2].rearrange("a d f -> (a d) f")
                src = src.rearrange("p (h w) -> p h w", h=H)

                # xa rows r = x rows r (0..47); xb rows r = x rows 47+r.
                # f32 -> bf16 cast happens inline in the (SWDGE) DMA, so
                # HBM read traffic is unchanged but matmuls run at bf16
                # rate and SBUF input tiles halve. Big transfers keep
                # SWDGE at line rate; only the LAST block is split finer
                # so its compute can trail the end of the stream closely.
                xa = xapool.tile([128, 48, W], dt.bfloat16, tag="xa")
                xb = xbpool.tile([128, 49, W], dt.bfloat16, tag="xb")
                nc.gpsimd.dma_start(xa[:, 0:31, :], src[:, 0:31, :])
                nc.gpsimd.dma_start(xb[:, 0:31, :], src[:, 47:78, :])
                nc.gpsimd.dma_start(xa[:, 31:48, :], src[:, 31:48, :])
                nc.gpsimd.dma_start(xb[:, 31:49, :], src[:, 78:96, :])

                ot = opool.tile([128, 2 * HO // 4, WO], dt.float32)

                for h0, cnt in CHUNKS:
                    ps = pspool.tile([128, 5, W], dt.float32, tag="ps",
                                     name="ps")
                    for g in range(2):
                        xt = xa if g == 0 else xb
                        # tap order 1,2,0 so the shortened k=0 tap of
                        # (chunk 0, half 0) accumulates into rows the
                        # k=1 tap already initialized
                        for k in (1, 2, 0):
                            base = 2 * h0 - 1 + k if g == 0 else 2 * h0 + k
                            lo = 0
                            if base < 0:
                                lo = 1  # h'=0 k=0 tap is the zero pad
                                base += 2
                            rows = cnt - lo
                            rhs = xt[:, base:base + 2 * rows - 1:2, :]
                            out = ps[g * 64:(g + 1) * 64, lo:cnt, :]
                            lhsT = bt[:, (bi * 3 + k) * 64:
                                      (bi * 3 + k + 1) * 64]
                            nc.tensor.matmul(
                                out, lhsT, rhs,
                                start=(k == 1), stop=(k == 0),
                                tile_position=(0, 64 * g) if g else None)

                    # W pass: out = left + r1*mid + r2*right, stride 2.
                    # Each op reads exactly one PSUM operand (HW limit);
                    # the w'=0 edge (zero left pad) falls out naturally.
                    orows = ot[:, h0:h0 + cnt, :]
                    pv = ps[:, 0:cnt, :]
                    # mid*r1 on ScalarE (own SBUF port; a DVE tensor_scalar
                    # could enter 2-port perf mode and starve SWDGE
                    # descriptor generation for the input cast-DMAs)
                    nc.scalar.activation(
                        orows[:, :, 0:WO], pv[:, :, 0:2 * WO:2],
                        mybir.ActivationFunctionType.Identity, scale=r1)
                    nc.vector.tensor_tensor(
                        orows[:, :, 1:WO], pv[:, :, 1:2 * WO - 2:2],
                        orows[:, :, 1:WO], add)
                    nc.vector.scalar_tensor_tensor(
                        orows[:, :, 0:WO], pv[:, :, 1:2 * WO:2], r2,
                        orows[:, :, 0:WO], mult, add)

                # one 128-partition output DMA per block on the SWDGE
                # (gpsimd) path: partitions (g, ncl, d'), 4608 B each
                # output in two pieces so chunks 0-2 rows leave while
                # chunks 3-4 still compute
                dst = y[b].rearrange("g a d (h w) -> (g a d) h w",
                                     h=HO // 2)
                nc.sync.dma_start(dst[:, 0:15, :], ot[:, 0:15, :])
                nc.sync.dma_start(dst[:, 15:24, :], ot[:, 15:24, :])
    nc.compile()
    return nc


def kernel(x, filt):
    x = np.ascontiguousarray(np.asarray(x, dtype=np.float32))
    filt = np.asarray(filt, dtype=np.float32)
    assert x.shape == (N, C, D, H, W), x.shape

    from concourse.bass_utils import run_bass_kernel_spmd

    dvec, hvec, wvec = _rank1_factors(filt)
    w0 = wvec[:, 0].copy()
    if not (np.abs(w0) > 1e-30).all():
        raise ValueError("W-tap pivot is zero; unsupported filter")
    r1 = wvec[:, 1] / w0
    r2 = wvec[:, 2] / w0

    uniform = bool(np.all(filt == filt[:1]))
    nbm = 1 if uniform else BLOCKS
    xr = x.reshape(N * C, D, H * W)

    in_maps = []
    for core in range(NCORES):
        chans = (np.arange(NC_PER_CORE) + core * NC_PER_CORE) % C
        # band matrices: rows (ncl*64 + d), col block (bi*3 + k),
        # cols (ncl*32 + d'); value dvec[delta]*hvec[k]*w0
        bm = np.zeros((128, nbm * 3 * 64), np.float32)
        # W-pass scalars per partition (g, ncl, d')
        wtp = np.empty((128, 2 * nbm), np.float32)
        for bi in range(nbm):
            for ncl in range(2):
                c = chans[2 * bi + ncl]
                for g in range(2):
                    rows = slice(g * 64 + ncl * 32, g * 64 + ncl * 32 + 32)
                    wtp[rows, 2 * bi] = r1[c]
                    wtp[rows, 2 * bi + 1] = r2[c]
                for k in range(3):
                    col0 = (bi * 3 + k) * 64 + ncl * 32
                    for dp in range(DO):
                        for delta in range(3):
                            d = 2 * dp - 1 + delta
                            if 0 <= d < D:
                                bm[ncl * 64 + d, col0 + dp] = (
                                    dvec[c, delta] * hvec[c, k] * w0[c])
        import ml_dtypes
        in_maps.append({
            "x": np.ascontiguousarray(
                xr[core * NC_PER_CORE:(core + 1) * NC_PER_CORE]),
            "bmat": bm.astype(ml_dtypes.bfloat16),
            "wtaps": wtp,
        })

    key = ("prog", uniform)
    if key not in _PROGRAM_CACHE:
        _PROGRAM_CACHE[key] = _build_program(uniform)
    nc = _PROGRAM_CACHE[key]

    trace = bool(int(os.environ.get("BLURPOOL_TRACE", "0")))
    kwargs = {}
    if trace and os.environ.get("BLURPOOL_TRACE_DIR"):
        kwargs["tmpdir"] = os.environ["BLURPOOL_TRACE_DIR"]
    res = run_bass_kernel_spmd(nc, in_maps, core_ids=list(range(NCORES)),
                               trace=trace, **kwargs)
    if trace:
        kernel.last_result = res

    parts = []
    for r in res.results:
        yg = r["y"].reshape(BLOCKS, 2, 2, DO, HO // 2, WO)
        # (b, g, ncl, d', h', w') -> (b, ncl, d', g, h', w')
        parts.append(yg.transpose(0, 2, 3, 1, 4, 5).reshape(
            NC_PER_CORE, DO, HO, WO))
    out = np.concatenate(parts, axis=0)
    return np.ascontiguousarray(out.reshape(N, C, DO, HO, WO))


# revision 26
# speedup vs baseline: 1.0122x; 1.0038x over previous
"""BlurPool3D Trainium2 kernel (8 cores, depthwise 3x3x3 blur, stride 2).

x[2, 64, 64, 96, 96] f32 -> y[2, 64, 32, 48, 48] f32. Rank-1 separable
filter (binomial [1,2,1]^3 / 64).

Strategy (v2 — subsample-first pass order):
  - Shard the 128 (n, c) pairs across 8 cores: 16 per core, 8 blocks of
    2 channels. Channels are independent (depthwise) -> no collectives.
  - Per block, SBUF partitions = (2 nc x 64 d) = 128; free = (h, w).
    Two input tiles per block: rows 0-47 and rows 47-95 (the two output
    h-halves need x rows [-1..47] / [47..95]).
  - D and H blur+subsample run FIRST, fused in one matmul family:
    lhsT = block-diagonal D-band matrix [128, 64] scaled by hvec[k]*w0,
    rhs = x rows (2h'-1+k) strided; 3 taps (k) accumulate in PSUM.
    float32r matmuls run at ~1 cycle/row (vs 4 for plain fp32) for
    N >= 256. Output: [64 (ncl,d'), 5h' x 96w] per PSUM bank; the two
    h-halves go to PE column groups 0/1 -> PSUM partitions 0-63/64-127.
    The h'=0 top edge (zero pad) is a shortened k=0 matmul; d-edges live
    in the band matrix. No memsets, no halo exchange.
  - W blur+subsample runs LAST on VectorE, on data already 4x smaller
    than x: per chunk two strided STTs (out = left + r1*mid + r2*right)
    reading PSUM directly, writing the SBUF out tile, plus a tiny
    2-op w'=0 edge column fix.
  - DMA: per block two ~2.4 MB input DMAs (sync=SP ring / scalar=ACT
    ring) and one 590 KB 128-partition output DMA on gpsimd (SWDGE) so
    output issue never head-blocks the HWDGE input streams.
"""

import os
import sys

for _p in ("/opt/trn_rl_repo",):
    if _p not in sys.path and os.path.isdir(_p):
        sys.path.insert(0, _p)

import numpy as np

N, C, D, H, W = 2, 64, 64, 96, 96
DO, HO, WO = 32, 48, 48
NCORES = 8
NC_PER_CORE = (N * C) // NCORES  # 16
BLOCKS = NC_PER_CORE // 2  # 8 blocks of 2 channels each

# h' chunks per h-half: PSUM bank = 512 f32 -> at most 5 rows of 96
CHUNKS = [(0, 5), (5, 5), (10, 5), (15, 5), (20, 4)]

_PROGRAM_CACHE = {}


def _rank1_factors(filt):
    """Per-channel rank-1 factorization filt[c,0] = outer(d, h, w)."""
    dvec = np.empty((C, 3), np.float64)
    hvec = np.empty((C, 3), np.float64)
    wvec = np.empty((C, 3), np.float64)
    for c in range(C):
        T = filt[c, 0].astype(np.float64)
        idx = np.unravel_index(np.argmax(np.abs(T)), T.shape)
        i0, j0, k0 = idx
        piv = T[i0, j0, k0]
        if piv == 0.0:
            dvec[c] = hvec[c] = wvec[c] = 0.0
            continue
        dvec[c] = T[:, j0, k0]
        hvec[c] = T[i0, :, k0] / piv
        wvec[c] = T[i0, j0, :] / piv
        recon = np.einsum("i,j,k->ijk", dvec[c], hvec[c], wvec[c])
        resid = np.abs(recon - T).max()
        if resid > 1e-6 * max(np.abs(T).max(), 1e-30):
            raise ValueError(f"filter channel {c} is not rank-1 (resid {resid})")
    return dvec, hvec, wvec


def _build_program(uniform):
    import concourse.bacc as bacc
    import concourse.mybir as mybir
    from concourse import tile

    dt = mybir.dt
    nc = bacc.Bacc("TRN2", target_bir_lowering=False, debug=False,
                   num_devices=NCORES)

    nbm = 1 if uniform else BLOCKS
    x = nc.dram_tensor("x", [NC_PER_CORE, D, H * W], dt.float32,
                       kind="ExternalInput")
    bmat = nc.dram_tensor("bmat", [128, nbm * 3 * 64], dt.bfloat16,
                          kind="ExternalInput")
    wtaps = nc.dram_tensor("wtaps", [128, 2 * nbm], dt.float32,
                           kind="ExternalInput")
    # block-major output layout: [block, h-half, ncl, d', 24*48] so each
    # block's output is one contiguous [128, 1152] DMA; host reassembles
    y = nc.dram_tensor("y", [BLOCKS, 2, 2, DO, (HO // 2) * WO], dt.float32,
                       kind="ExternalOutput")

    mult = mybir.AluOpType.mult
    add = mybir.AluOpType.add

    with tile.TileContext(nc) as tc:
        with tc.tile_pool(name="const", bufs=1) as cpool, \
             tc.tile_pool(name="xa", bufs=5) as xapool, \
             tc.tile_pool(name="xb", bufs=5) as xbpool, \
             tc.tile_pool(name="op", bufs=4) as opool, \
             tc.tile_pool(name="ps", bufs=8, space="PSUM") as pspool:
            bt = cpool.tile([128, nbm * 3 * 64], dt.bfloat16)
            wt = cpool.tile([128, 2 * nbm], dt.float32)
            nc.sync.dma_start(bt[:], bmat[:])
            nc.sync.dma_start(wt[:], wtaps[:])

            for b in range(BLOCKS):
                bi = 0 if uniform else b
                r1 = wt[:, 2 * bi:2 * bi + 1]
                r2 = wt[:, 2 * bi + 1:2 * bi + 2]
                src = x[2 * b:2 * b + 2].rearrange("a d f -> (a d) f")
                src = src.rearrange("p (h w) -> p h w", h=H)

                # xa rows r = x rows r (0..47); xb rows r = x rows 47+r.
                # f32 -> bf16 cast happens inline in the (SWDGE) DMA, so
                # HBM read traffic is unchanged but matmuls run at bf16
                # rate and SBUF input tiles halve. Big transfers keep
                # SWDGE at line rate; only the LAST block is split finer
                # so its compute can trail the end of the stream closely.
                xa = xapool.tile([128, 48, W], dt.bfloat16, tag="xa")
                xb = xbpool.tile([128, 49, W], dt.bfloat16, tag="xb")
                if b == 0:
                    # first block: big DMAs -- fewer serial Q7 descriptor
                    # generations before the stream saturates
                    nc.gpsimd.dma_start(xa[:], src[:, 0:48, :])
                    nc.gpsimd.dma_start(xb[:], src[:, 47:96, :])
                elif b < BLOCKS - 1:
                    nc.gpsimd.dma_start(xa[:, 0:31, :], src[:, 0:31, :])
                    nc.gpsimd.dma_start(xb[:, 0:31, :], src[:, 47:78, :])
                    nc.gpsimd.dma_start(xa[:, 31:48, :], src[:, 31:48, :])
                    nc.gpsimd.dma_start(xb[:, 31:49, :], src[:, 78:96, :])
                else:
                    # last block: 3 pieces per half so the final chunks
                    # trail the end of the stream as closely as possible
                    nc.gpsimd.dma_start(xa[:, 0:21, :], src[:, 0:21, :])
                    nc.gpsimd.dma_start(xb[:, 0:21, :], src[:, 47:68, :])
                    nc.gpsimd.dma_start(xa[:, 21:41, :], src[:, 21:41, :])
                    nc.gpsimd.dma_start(xb[:, 21:41, :], src[:, 68:88, :])
                    nc.gpsimd.dma_start(xa[:, 41:48, :], src[:, 41:48, :])
                    nc.gpsimd.dma_start(xb[:, 41:49, :], src[:, 88:96, :])

                ot = opool.tile([128, 2 * HO // 4, WO], dt.float32)

                for h0, cnt in CHUNKS:
                    ps = pspool.tile([128, 5, W], dt.float32, tag="ps",
                                     name="ps")
                    for g in range(2):
                        xt = xa if g == 0 else xb
                        # tap order 1,2,0 so the shortened k=0 tap of
                        # (chunk 0, half 0) accumulates into rows the
                        # k=1 tap already initialized
                        for k in (1, 2, 0):
                            base = 2 * h0 - 1 + k if g == 0 else 2 * h0 + k
                            lo = 0
                            if base < 0:
                                lo = 1  # h'=0 k=0 tap is the zero pad
                                base += 2
                            rows = cnt - lo
                            rhs = xt[:, base:base + 2 * rows - 1:2, :]
                            out = ps[g * 64:(g + 1) * 64, lo:cnt, :]
                            lhsT = bt[:, (bi * 3 + k) * 64:
                                      (bi * 3 + k + 1) * 64]
                            nc.tensor.matmul(
                                out, lhsT, rhs,
                                start=(k == 1), stop=(k == 0),
                                tile_position=(0, 64 * g) if g else None)

                    # W pass: out = left + r1*mid + r2*right, stride 2.
                    # Each op reads exactly one PSUM operand (HW limit);
                    # the w'=0 edge (zero left pad) falls out naturally.
                    orows = ot[:, h0:h0 + cnt, :]
                    pv = ps[:, 0:cnt, :]
                    # mid*r1 on ScalarE (own SBUF port; a DVE tensor_scalar
                    # could enter 2-port perf mode and starve SWDGE
                    # descriptor generation for the input cast-DMAs)
                    nc.scalar.activation(
                        orows[:, :, 0:WO], pv[:, :, 0:2 * WO:2],
                        mybir.ActivationFunctionType.Identity, scale=r1)
                    nc.vector.tensor_tensor(
                        orows[:, :, 1:WO], pv[:, :, 1:2 * WO - 2:2],
                        orows[:, :, 1:WO], add)
                    nc.vector.scalar_tensor_tensor(
                        orows[:, :, 0:WO], pv[:, :, 1:2 * WO:2], r2,
                        orows[:, :, 0:WO], mult, add)

                # one 128-partition output DMA per block on the SWDGE
                # (gpsimd) path: partitions (g, ncl, d'), 4608 B each
                # output in pieces so early chunks' rows leave while later
                # chunks still compute (3 pieces for the last block)
                dst = y[b].rearrange("g a d (h w) -> (g a d) h w",
                                     h=HO // 2)
                if b < BLOCKS - 1:
                    nc.sync.dma_start(dst[:, 0:15, :], ot[:, 0:15, :])
                    nc.sync.dma_start(dst[:, 15:24, :], ot[:, 15:24, :])
                else:
                    nc.sync.dma_start(dst[:, 0:10, :], ot[:, 0:10, :])
                    nc.sync.dma_start(dst[:, 10:20, :], ot[:, 10:20, :])
                    nc.sync.dma_start(dst[:, 20:24, :], ot[:, 20:24, :])
    nc.compile()
    return nc


def kernel(x, filt):
    x = np.ascontiguousarray(np.asarray(x, dtype=np.float32))
    filt = np.asarray(filt, dtype=np.float32)
    assert x.shape == (N, C, D, H, W), x.shape

    from concourse.bass_utils import run_bass_kernel_spmd

    dvec, hvec, wvec = _rank1_factors(filt)
    w0 = wvec[:, 0].copy()
    if not (np.abs(w0) > 1e-30).all():
        raise ValueError("W-tap pivot is zero; unsupported filter")
    r1 = wvec[:, 1] / w0
    r2 = wvec[:, 2] / w0

    uniform = bool(np.all(filt == filt[:1]))
    nbm = 1 if uniform else BLOCKS
    xr = x.reshape(N * C, D, H * W)

    in_maps = []
    for core in range(NCORES):
        chans = (np.arange(NC_PER_CORE) + core * NC_PER_CORE) % C
        # band matrices: rows (ncl*64 + d), col block (bi*3 + k),
        # cols (ncl*32 + d'); value dvec[delta]*hvec[k]*w0
        bm = np.zeros((128, nbm * 3 * 64), np.float32)
        # W-pass scalars per partition (g, ncl, d')
        wtp = np.empty((128, 2 * nbm), np.float32)
        for bi in range(nbm):
            for ncl in range(2):
                c = chans[2 * bi + ncl]
                for g in range(2):
                    rows = slice(g * 64 + ncl * 32, g * 64 + ncl * 32 + 32)
                    wtp[rows, 2 * bi] = r1[c]
                    wtp[rows, 2 * bi + 1] = r2[c]
                for k in range(3):
                    col0 = (bi * 3 + k) * 64 + ncl * 32
                    for dp in range(DO):
                        for delta in range(3):
                            d = 2 * dp - 1 + delta
                            if 0 <= d < D:
                                bm[ncl * 64 + d, col0 + dp] = (
                                    dvec[c, delta] * hvec[c, k] * w0[c])
        import ml_dtypes
        in_maps.append({
            "x": np.ascontiguousarray(
                xr[core * NC_PER_CORE:(core + 1) * NC_PER_CORE]),
            "bmat": bm.astype(ml_dtypes.bfloat16),
            "wtaps": wtp,
        })

    key = ("prog", uniform)
    if key not in _PROGRAM_CACHE:
        _PROGRAM_CACHE[key] = _build_program(uniform)
    nc = _PROGRAM_CACHE[key]

    trace = bool(int(os.environ.get("BLURPOOL_TRACE", "0")))
    kwargs = {}
    if trace and os.environ.get("BLURPOOL_TRACE_DIR"):
        kwargs["tmpdir"] = os.environ["BLURPOOL_TRACE_DIR"]
    res = run_bass_kernel_spmd(nc, in_maps, core_ids=list(range(NCORES)),
                               trace=trace, **kwargs)
    if trace:
        kernel.last_result = res

    parts = []
    for r in res.results:
        yg = r["y"].reshape(BLOCKS, 2, 2, DO, HO // 2, WO)
        # (b, g, ncl, d', h', w') -> (b, ncl, d', g, h', w')
        parts.append(yg.transpose(0, 2, 3, 1, 4, 5).reshape(
            NC_PER_CORE, DO, HO, WO))
    out = np.concatenate(parts, axis=0)
    return np.ascontiguousarray(out.reshape(N, C, DO, HO, WO))
